# revision 19
# baseline (speedup 1.0000x reference)
"""Trainium2 Bass kernel: 7x7 valid cross-correlation (Conv2D) + bias on a
4096x4096 fp32 image, column-strip sharded over 8 NeuronCores (512 output
cols each, with a 6-col halo in each core's input strip).

Algorithm per core:
  - Output rows are processed in 34 tiles of 122 (=128-6) rows; each tile is
    one 512-wide PSUM chunk covering the core's whole column strip.
  - The 2D conv is 7 horizontal taps b, each an accumulating TensorE matmul:
        psum[m, n] += B_b.T @ x[:, n+b]
    where B_b[k, m] = w[k-m, b] is a banded [128 x 122] matrix performing the
    7-tap vertical convolution for kernel column b.
  - One adjacent tap pair (chosen at runtime as the min-|w|^2 adjacent column
    pair, so its quantization error is smallest) runs as a single fp8-e4m3
    DoubleRow matmul: the PE virtualizes to 128x256 and computes
    A_w.T@A_i + B_w.T@B_i in ~0.55x the time of two bf16 matmuls. The two
    ifmaps are fp8 copies of the x strip shifted by 0 and 1 columns, laid
    out as planes [128, 2, 512] so the pair-dim step (512B) satisfies the
    DoubleRow step%16 constraint. Measured rel err ~1.6e-2 (< 2e-2 budget);
    remaining 5 taps stay bf16.
  - Head: x0 rides the gpsimd SWDGE queue while the two HWDGE queues carry
    the (bf16-taps-only) band + x1 halves, balancing the three queues'
    early delivery so real work starts ~11.5us. Warmup matmuls on a
    memset tile bridge PE engine-ready (~7.3us) to first-data so the HAM
    activity window flips during warmup and real matmuls run at full clock
    from the first tile.
  - fp8 x planes are prefetched 12 tiles ahead on the SWDGE queue: far
    enough to never starve the PE, shallow enough that the ~0.65us/DMA
    descriptor posts don't delay the y4 store posts (in-order queue).
  - Stores: groups of 4 tiles to fully-contiguous 0.5MB DRAM bursts on
    SWDGE; the trailing 2/2/1/1-tile groups go contiguously via the HWDGE
    queues (split by partition halves across both), emitted after the tile
    loop so they can't head-of-line block x loads, keeping the
    end-of-kernel drain short.
  - PSUM is evacuated by VectorE with a fused bias add + bf16 downcast.
  - Measured: ~62us HW exec (baseline 76us), rel err 1.58e-2.
"""

import sys

sys.path.insert(0, "/opt/trn_rl_repo")

import ml_dtypes
import numpy as np

import concourse.bass as bass
import concourse.bacc as bacc
import concourse.mybir as mybir
from concourse.tile import TileContext
from concourse.bass_utils import run_bass_kernel_spmd

KH, KW = 7, 7
H, W = 4096, 4096
OH, OW = H - KH + 1, W - KW + 1  # 4090, 4090

NCORES = 8
CORE_OC = 512                    # output cols per core (core 7: 506 valid)
CORE_IC = CORE_OC + KH - 1       # 518 input cols needed
CORE_IC_PAD = 528                # pad rows to 1056B (32B-aligned, contiguous)
TILE_R = 128 - (KH - 1)          # 122 output rows per row-tile
N_TILES = -(-OH // TILE_R)       # 34
# Tiles 0-27 store as 4-tile contiguous SWDGE bursts; tiles 28-31 as two
# 2-tile groups and 32-33 as two 1-tile groups, all via the HWDGE queues
# (split by partition halves), so the SWDGE queue drains well before the
# end of the kernel instead of gating it.
GROUPS = [4] * 7 + [2, 2] + [1, 1]  # tiles per store group (sum = 34)
N_G4 = sum(1 for g in GROUPS if g == 4)
N_G2 = sum(1 for g in GROUPS if g == 2)
N_G1 = sum(1 for g in GROUPS if g == 1)
WARMUP_MMS = 9
USE_FP8 = True

BF16 = ml_dtypes.bfloat16
F8E4 = ml_dtypes.float8_e4m3fn

_NC_CACHE = {}


def _build_nc(pair):
    """pair: index p of the adjacent horizontal-tap pair (p, p+1) computed as
    one fp8 DoubleRow matmul, or None for all-bf16."""
    f32 = mybir.dt.float32
    bf16 = mybir.dt.bfloat16
    f8 = mybir.dt.float8e4
    kin = TILE_R + KH - 1  # 128
    assert kin == 128
    taps_bf = [b for b in range(KW) if pair is None or b not in (pair, pair + 1)]

    nc = bacc.Bacc()
    nbf = len(taps_bf)
    x_in = nc.declare_dram_parameter("x_in", [H, CORE_IC_PAD], bf16, isOutput=False)
    # Only the bf16 taps' band blocks ship (the fp8 pair lives in band8):
    # 152KB instead of 218KB on the head-critical HWDGE queues.
    bands = nc.declare_dram_parameter("bands", [kin, nbf * TILE_R], bf16, isOutput=False)
    biasb = nc.declare_dram_parameter("biasb", [128, 1], f32, isOutput=False)
    if pair is not None:
        # fp8 planes: x8_in[r, i, j] = fp8(x[r, pair+i+j]), i in {0,1}
        x8_in = nc.declare_dram_parameter("x8_in", [H, 2, CORE_OC], f8, isOutput=False)
        band8 = nc.declare_dram_parameter("band8", [kin, 2, 128], f8, isOutput=False)
    # Partition-major per group: y4[g, p, j*OC:(j+1)*OC] = output row
    # TILE_R*(4g+j) + p; y1[i, p, :] = output row TILE_R*(32+i) + p.
    # Every store covers a fully contiguous DRAM range (host unscrambles).
    y4 = nc.declare_dram_parameter("y4", [N_G4, 128, 4 * CORE_OC], bf16, isOutput=True)
    y2 = nc.declare_dram_parameter("y2", [N_G2, 128, 2 * CORE_OC], bf16, isOutput=True)
    y1 = nc.declare_dram_parameter("y1", [N_G1, 128, CORE_OC], bf16, isOutput=True)

    with TileContext(nc) as tc:
        with (
            tc.tile_pool(name="const", bufs=1) as cpool,

            tc.tile_pool(name="xio", bufs=12) as xpool,
            tc.tile_pool(name="x8io", bufs=16) as x8pool,
            tc.tile_pool(name="yio", bufs=6) as ypool,
            tc.tile_pool(name="ps", bufs=8, space="PSUM") as ppool,
        ):
            band_sb = cpool.tile([kin, nbf * TILE_R], bf16)
            bias_sb = cpool.tile([128, 1], f32)
            warm_sb = cpool.tile([128, CORE_OC], bf16)
            warm_ps = ppool.tile([128, CORE_OC], f32, tag="ps")

            # Constants go first on the fast HWDGE queues, band split across
            # both by partition halves (109KB each).
            nc.sync.dma_start(out=band_sb[:64, :], in_=bands[:64, :])
            nc.scalar.dma_start(out=band_sb[64:, :], in_=bands[64:, :])
            nc.sync.dma_start(out=bias_sb[:, :], in_=biasb[:, :])

            # Tile 0's bf16 strip rides the SWDGE queue, which starts
            # delivering ~2us before the HWDGE queues work through the band:
            # with the bf16 taps ordered first, real work starts ~11us.
            x0_sb = xpool.tile([kin, CORE_IC_PAD], bf16, tag="x")
            nc.gpsimd.dma_start(out=x0_sb[:, :], in_=x_in[:128, :])
            def _post_x8(t):
                r0 = t * TILE_R
                kh = min(TILE_R, OH - r0) + KH - 1
                x8_sb = x8pool.tile([kin, 2, CORE_OC], f8, tag="x8")
                nc.gpsimd.dma_start(out=x8_sb[:kh, :, :], in_=x8_in[r0 : r0 + kh, :, :])
                x8_tiles.append(x8_sb)

            if pair is not None:
                band8_sb = cpool.tile([kin, 2, 128], f8)
                nc.gpsimd.dma_start(out=band8_sb[:, :, :], in_=band8[:, :, :])
                # Prefetch the fp8 plane loads 12 tiles ahead: each SWDGE
                # descriptor post costs the gpsimd engine ~0.65us, so posting
                # all 34 upfront would delay the y4 store posts by ~20us and
                # stall evacs on y-buffer reuse; 12-ahead keeps the loads
                # safely early while letting store posts interleave.
                x8_tiles = []
                for t in range(min(12, N_TILES)):
                    _post_x8(t)

            # Vector is the earliest-free engine; memset there so warmups
            # start at PE engine-ready and keep the HAM activity window
            # alive until the first real matmul's data lands (~11.5us).
            nc.vector.memset(warm_sb[:, :], 0)
            for _ in range(WARMUP_MMS):
                nc.tensor.matmul(
                    warm_ps[:, :],
                    lhsT=warm_sb[:, :128],
                    rhs=warm_sb[:, :],
                    start=True,
                    stop=True,
                )

            t = 0
            g4 = 0
            g2 = 0
            g1 = 0
            deferred = []
            for gsz in GROUPS:
                y_sb = ypool.tile([128, 4 * CORE_OC], bf16, tag="y")
                for j in range(gsz):
                    r0 = t * TILE_R
                    h = min(TILE_R, OH - r0)
                    kh = h + KH - 1
                    if pair is not None and t + 12 < N_TILES:
                        _post_x8(t + 12)
                    if t == 0:
                        x_sb = x0_sb
                    else:
                        x_sb = xpool.tile([kin, CORE_IC_PAD], bf16, tag="x")
                    if t == 0:
                        pass  # loaded via SWDGE above
                    elif t == 1:
                        # tile 1 gates the pipeline start: split across both
                        # HWDGE queues to halve its in-flight latency.
                        nc.sync.dma_start(out=x_sb[:64, :], in_=x_in[r0 : r0 + 64, :])
                        nc.scalar.dma_start(
                            out=x_sb[64:kh, :], in_=x_in[r0 + 64 : r0 + kh, :]
                        )
                    else:
                        ldq = nc.sync if t % 2 == 0 else nc.scalar
                        ldq.dma_start(out=x_sb[:kh, :], in_=x_in[r0 : r0 + kh, :])
                    ps = ppool.tile([128, CORE_OC], f32, tag="ps")
                    for i, b in enumerate(taps_bf):
                        nc.tensor.matmul(
                            ps[:h, :],
                            lhsT=band_sb[:kh, i * TILE_R : i * TILE_R + h],
                            rhs=x_sb[:kh, b : b + CORE_OC],
                            start=(i == 0),
                            stop=(pair is None and i == len(taps_bf) - 1),
                        )
                    if pair is not None:
                        nc.tensor.matmul(
                            ps[:h, :],
                            lhsT=band8_sb[:kh, :, :h],
                            rhs=x8_tiles[t][:kh, :, :],
                            start=False,
                            stop=True,
                            perf_mode=mybir.MatmulPerfMode.DoubleRow,
                        )
                    nc.vector.tensor_scalar_add(
                        y_sb[:h, j * CORE_OC : (j + 1) * CORE_OC],
                        ps[:h, :],
                        bias_sb[:h, 0:1],
                    )
                    t += 1
                # All 128 partitions are stored even for partial tiles:
                # partitions 0-63 and 64-127 map to different SDMA engines,
                # so a 64-row store would serialize onto a single engine.
                # Trailing HWDGE stores are deferred until after the tile
                # loop: the queues are in-order, and an evac-gated store
                # emitted mid-loop would head-of-line block later x loads.
                if gsz == 4:
                    nc.gpsimd.dma_start(out=y4[g4, :, :], in_=y_sb[:, :])
                    g4 += 1
                elif gsz == 2:
                    deferred.append(('y2', y2[g2, :, :], y_sb, 2 * CORE_OC))
                    g2 += 1
                else:
                    deferred.append(('y1', y1[g1, :, :], y_sb, CORE_OC))
                    g1 += 1
            for kind, dst, y_sb, w in deferred:
                nc.sync.dma_start(out=dst[:64, :], in_=y_sb[:64, :w])
                nc.scalar.dma_start(out=dst[64:, :], in_=y_sb[64:, :w])
    nc.compile()
    return nc


def _make_bands(weight, taps):
    """B_b[k, m] = w[k-m, b] laid out as [kin, len(taps)*TILE_R] (the i-th
    block holds band taps[i])."""
    kin = TILE_R + KH - 1
    bands = np.zeros((kin, len(taps) * TILE_R), np.float32)
    m = np.arange(TILE_R)
    for i, b in enumerate(taps):
        for a in range(KH):
            bands[m + a, i * TILE_R + m] = weight[a, b]
    return bands.astype(BF16)


def _make_band8(weight, pair):
    kin = TILE_R + KH - 1
    band8 = np.zeros((kin, 2, 128), np.float32)
    m = np.arange(TILE_R)
    for i in range(2):
        for a in range(KH):
            band8[m + a, i, m] = weight[a, pair + i]
    return band8.astype(F8E4)


def _pick_pair(weight):
    col2 = (np.asarray(weight, np.float64) ** 2).sum(axis=0)
    pair_cost = col2[:-1] + col2[1:]
    return int(np.argmin(pair_cost))


def _shard_inputs(x, weight, bias, pair):
    taps = [b for b in range(KW) if pair is None or b not in (pair, pair + 1)]
    bands = _make_bands(weight, taps)
    biasb = np.full((128, 1), np.float32(bias[0]), np.float32)
    xb = x.astype(BF16)
    if pair is not None:
        band8 = _make_band8(weight, pair)
        x8 = x.astype(F8E4)
    in_maps = []
    for c in range(NCORES):
        c0 = c * CORE_OC
        cc = min(CORE_IC, W - c0)
        xt = np.zeros((H, CORE_IC_PAD), BF16)
        xt[:, :cc] = xb[:, c0 : c0 + cc]
        m = {"x_in": xt, "bands": bands, "biasb": biasb}
        if pair is not None:
            x8t = np.zeros((H, 2, CORE_OC), F8E4)
            for i in range(2):
                s0 = c0 + pair + i
                sc = max(0, min(CORE_OC, W - s0))
                x8t[:, i, :sc] = x8[:, s0 : s0 + sc]
            m["x8_in"] = x8t
            m["band8"] = band8
        in_maps.append(m)
    return in_maps


def _assemble(results):
    out = np.empty((OH, OW), np.float32)
    for c in range(NCORES):
        c0 = c * CORE_OC
        cw = min(CORE_OC, OW - c0)
        y4 = results[c]["y4"]  # [N_G4, 128, 4*CORE_OC]
        y2 = results[c]["y2"]  # [N_G2, 128, 2*CORE_OC]
        y1 = results[c]["y1"]  # [N_G1, 128, CORE_OC]
        strip = np.empty((OH, CORE_OC), np.float32)
        t = 0
        g4 = 0
        g2 = 0
        g1 = 0
        for gsz in GROUPS:
            for j in range(gsz):
                r0 = t * TILE_R
                h = min(TILE_R, OH - r0)
                if gsz == 4:
                    strip[r0 : r0 + h, :] = y4[g4, :h, j * CORE_OC : (j + 1) * CORE_OC]
                elif gsz == 2:
                    strip[r0 : r0 + h, :] = y2[g2, :h, j * CORE_OC : (j + 1) * CORE_OC]
                else:
                    strip[r0 : r0 + h, :] = y1[g1, :h, :]
                t += 1
            if gsz == 4:
                g4 += 1
            elif gsz == 2:
                g2 += 1
            else:
                g1 += 1
        out[:, c0 : c0 + cw] = strip[:, :cw]
    return out


def _get_nc(pair):
    key = (CORE_OC, TILE_R, WARMUP_MMS, pair)
    if key not in _NC_CACHE:
        _NC_CACHE[key] = _build_nc(pair)
    return _NC_CACHE[key]


def _run(x, weight, bias, **spmd_kwargs):
    x = np.ascontiguousarray(np.asarray(x), dtype=np.float32)
    weight = np.asarray(weight, dtype=np.float32)
    bias = np.asarray(bias, dtype=np.float32)
    pair = _pick_pair(weight) if USE_FP8 else None
    in_maps = _shard_inputs(x, weight, bias, pair)
    try:
        res = run_bass_kernel_spmd(
            _get_nc(pair), in_maps, list(range(NCORES)), **spmd_kwargs
        )
    except Exception:
        # One retry: a freshly-opened device occasionally reports a
        # transient NRT_EXEC_UNIT_UNRECOVERABLE on the first execution;
        # re-running on the re-opened device succeeds.
        res = run_bass_kernel_spmd(
            _get_nc(pair), in_maps, list(range(NCORES)), **spmd_kwargs
        )
    return _assemble(res.results), res


def kernel(x, weight, bias):
    out, _ = _run(x, weight, bias)
    return out
